# revision 25
# baseline (speedup 1.0000x reference)
"""Trainium2 Bass kernel for nn_LocalAttention (depthwise causal conv + RoPE +
windowed local attention), data-parallel over the batch dim on 8 NeuronCores.

Self-contained: hardcodes shapes B=32, N=4096, D=64, WS=128 and the sharding
(4 batches per core). Host-side prep is limited to dtype casts and layout
transforms (Toeplitz band tables from the depthwise conv weights, RoPE cos/sin
tables, tiling of x into the SBUF partition layout); all FLOPs over the
activations run on device.

Layout/scheduling notes (349us baseline -> ~226us):
- All DRAM inputs are pre-tiled on host so every DMA is [128 partitions x
  big-contiguous-run]: ~2.5K descriptors total instead of ~115K, and the
  load order (xq + toeplitz chunks on the sync queue, xk/xv/tables on the
  scalar queue) lets conv-q start at ~10us instead of ~36us.
- qc/kc/vsb use (d, w, b)-style column layouts so every conv PSUM drain is
  a contiguous [128, 1024] copy (strided scatters cost 4x on DVE/ACT).
- The softmax-denominator ones-column is folded into V (65-wide AV
  matmuls); AV PSUM groups are 4 windows padded to one 2KB bank (a
  [128, 520] f32 tile crosses a bank boundary mid-matmul and corrupts).
- Attention runs per-4-window-group (sim -> exp -> causal-mask -> AV ->
  scaled drain) so ACT/DVE/Pool pipeline behind the PE within a batch.
- Activation engine does exp + some drains; causal mask on Pool/DVE.
- Output is written bf16 in on-chip layout and fixed up on host.
- Remaining wall time is PE-bound: this toolchain serializes a ~M/1.2GHz
  LDWEIGHTS before every matmul (enable-ldw-opt=false, incompatible with
  the explicit InstLdweights bass emits), so each matmul costs about
  (M + N) * 0.83ns and the conv/transpose/sim/AV total is ~244K col-units.
"""

import sys

sys.path.insert(0, "/opt/trn_rl_repo")

import ml_dtypes
import numpy as np

import concourse.bass as bass
import concourse.mybir as mybir
import concourse.tile as tile
from concourse.bass_utils import run_bass_kernel_spmd
from concourse.masks import make_identity


BF16 = mybir.dt.bfloat16
F32 = mybir.dt.float32
NPBF = ml_dtypes.bfloat16

B, N, D, WS = 32, 4096, 64, 128
W = N // WS              # 32 windows
NCORES = 8
BL = B // NCORES         # 4 batches per core
NWP = W + 1              # 33 window slots (slot 0 = zero pad = "window -1")
SCALE = D ** -0.5
XCOLS = NWP * BL * D     # 8448   x cols (w', b, d)
TCOLS = D * 2 * WS       # 16384  toeplitz cols (d, {hi,lo}, jout)
QCOLS = BL * D * W       # 8192   qc/kc cols (b, d, w)
VCOLS = BL * W * 65      # 8320   vsb cols (b, w, d+ones)
PCOLS = W * 2 * WS       # 8192   p cols (m, 256)
OTC = W * D              # 2048   out cols (w, d)
ROPE_BASE = 10000.0
DEBUG = False


def _split_multiwaits(nc, max_waits=1):
    """walrus in this env rejects >1 sem wait per instruction; split extras
    into standalone NoOp waits inserted just before, on the same engine."""
    n_fixed = 0
    for fn in nc.m.functions:
        for bb in fn.blocks:
            insts = bb.instructions
            new_list = []
            changed = False
            for inst in insts:
                si = inst.sync_info
                if si is not None and si.on_wait and len(si.on_wait) > max_waits:
                    waits = list(si.on_wait)
                    for w in waits[:-max_waits]:
                        nop = mybir.InstNoOp(
                            name=f"{inst.name}-xw{n_fixed}",
                            engine=inst.engine,
                            ins=[],
                            outs=[],
                            sync_info=mybir.SyncInfo(on_wait=[w], on_update=[]),
                        )
                        new_list.append(nop)
                        n_fixed += 1
                    si.on_wait = waits[-max_waits:]
                    changed = True
                new_list.append(inst)
            if changed:
                bb.instructions = new_list
    return n_fixed


def _ap(t, offset, dims):
    """AP over tile/dram tensor t: partition dim kept, free dims replaced."""
    return bass.AP(tensor=t.tensor, offset=t.offset + offset, ap=[t.ap[0]] + dims)


def _build_program():
    nc = bass.Bass()
    xd = {n: nc.dram_tensor(f"x{n}", [128, XCOLS], BF16, kind="ExternalInput")
          for n in ("q", "k", "v")}
    td = {n: nc.dram_tensor(f"t{n}", [128, TCOLS], BF16, kind="ExternalInput")
          for n in ("q", "k", "v")}
    cosb = nc.dram_tensor("cosb", [128, D * W], BF16, kind="ExternalInput")
    sinb = nc.dram_tensor("sinb", [128, D * W], BF16, kind="ExternalInput")
    out = nc.dram_tensor("out", [BL * 128, OTC], BF16, kind="ExternalOutput")
    dbg = {}
    if DEBUG:
        dbg["qc"] = nc.dram_tensor("dbg_qc", [128, QCOLS], BF16, kind="ExternalOutput")
        dbg["kc"] = nc.dram_tensor("dbg_kc", [128, QCOLS], BF16, kind="ExternalOutput")
        dbg["vsb"] = nc.dram_tensor("dbg_vsb", [128, VCOLS], BF16, kind="ExternalOutput")
        dbg["p0"] = nc.dram_tensor("dbg_p0", [128, PCOLS], BF16, kind="ExternalOutput")
        dbg["qt0"] = nc.dram_tensor("dbg_qt0", [128, W * WS], BF16, kind="ExternalOutput")
        dbg["kt0"] = nc.dram_tensor("dbg_kt0", [128, W * WS], BF16, kind="ExternalOutput")

    with tile.TileContext(nc) as tc:
        # ---------------- persistent pools
        const = tc.alloc_tile_pool(name="const", bufs=1)
        mid = tc.alloc_tile_pool(name="mid", bufs=1)

        ident = const.tile([128, 128], BF16, tag="ident")
        make_identity(nc, ident)
        tri = const.tile([128, 128], BF16, tag="tri")  # tri[j,i]=1 iff i>=j
        nc.vector.memset(tri[:], 1.0)
        nc.gpsimd.affine_select(
            out=tri[:], in_=tri[:], compare_op=mybir.AluOpType.is_ge,
            fill=0.0, base=0, channel_multiplier=-1, pattern=[[1, 128]],
        )
        costab = const.tile([128, D * W], BF16, tag="cos")
        nc.scalar.dma_start(out=costab[:], in_=cosb[:])
        sintab = const.tile([128, D * W], BF16, tag="sin")
        nc.scalar.dma_start(out=sintab[:], in_=sinb[:])

        qc = mid.tile([128, QCOLS], BF16, tag="qc")
        kc = mid.tile([128, QCOLS], BF16, tag="kc")
        vsb = mid.tile([128, VCOLS], BF16, tag="vsb")
        # ones block (channel 64) for the folded softmax denominator
        nc.vector.memset(vsb[:, 64 * 128: 65 * 128], 1.0)
        qt = {}
        for _n in ("q", "k"):
            for _b in range(BL):
                qt[(_n, _b)] = mid.tile([64, W * WS], BF16,
                                        tag=f"{_n}t{_b}", name=f"{_n}t{_b}")

        # ---------------- conv + rope + transpose phase
        xpool = tc.alloc_tile_pool(name="x", bufs=2)
        tpool = tc.alloc_tile_pool(name="toep", bufs=2)
        tmp = tc.alloc_tile_pool(name="tmp", bufs=2)
        convps = tc.alloc_tile_pool(name="convps", bufs=3, space="PSUM")
        tps = tc.alloc_tile_pool(name="tps", bufs=2, space="PSUM")

        # input loads: one big contiguous DMA per tensor, toeplitz in 4 chunks
        xt = {}
        for n in ("q", "k", "v"):
            t = xpool.tile([128, XCOLS], BF16, tag="x", name=f"x{n}")
            # xq on the sync queue ahead of the tq chunks; xk/xv go down the
            # scalar queue so they don't delay conv-q's toeplitz chunks
            eng = nc.sync if n == "q" else nc.scalar
            eng.dma_start(out=t[:], in_=xd[n][:])
            xt[n] = t

        drain_eng = [nc.vector.tensor_copy, nc.scalar.copy]

        # HAM warm-up: ~5us of dense junk matmuls gated on the xq arrival.
        # The PE idles for the DMA anyway; one sustained-busy SHORT window
        # flips the clock gate 1.2GHz -> 2.4GHz and the micro-gaps of the
        # real kernel never re-throttle it.
        wp = convps.tile([128, 1024], F32, tag="cp", name="warm")
        for i in range(10):
            nc.tensor.matmul(wp[:, 0:512], ident[:],
                             xt["q"][:, i * 512: (i + 1) * 512],
                             start=True, stop=True)
        # keep the burst live past DCE; the rope pass fully overwrites this
        wdrain = tmp.tile([128, QCOLS], BF16, tag="tmp", name="wdrain")
        nc.vector.tensor_copy(wdrain[:, 0:128], wp[:, 0:128])

        def conv(n):
            """Depthwise causal conv for tensor n via per-channel Toeplitz
            matmuls; drains into qc/kc (cols b,d,w) or vsb (cols b,w,d)."""
            x = xt[n]
            for c in range(4):           # 16-channel toeplitz chunks
                tt = tpool.tile([128, 16 * 256], BF16, tag="toep")
                nc.sync.dma_start(
                    out=tt[:],
                    in_=bass.AP(tensor=td[n], offset=c * 4096,
                                ap=[[TCOLS, 128], [1, 4096]]))
                for g2 in range(2):      # psum groups of 8 channels
                    g = c * 2 + g2
                    cp = convps.tile([128, 8 * 128], F32, tag="cp")
                    for dd in range(8):
                        d = g * 8 + dd
                        dloc = d % 16
                        hi = tt[:, dloc * 256: dloc * 256 + 128]
                        lo = tt[:, dloc * 256 + 128: dloc * 256 + 256]
                        # moving: x cols (w', b) at channel d
                        rhs_hi = _ap(x, BL * D + d, [[BL * D, W], [D, BL]])
                        rhs_lo = _ap(x, d, [[BL * D, W], [D, BL]])
                        ps = cp[:, dd * 128: (dd + 1) * 128]
                        nc.tensor.matmul(ps, hi, rhs_hi, start=True, stop=False)
                        nc.tensor.matmul(ps, lo, rhs_lo, start=False, stop=True)
                    # drain; cp is [128 jout, (8 d, 32 w, 4 b)] and the
                    # qc/kc/vsb layouts are (d, w, b) so the copy is contiguous
                    dstt = vsb if n == "v" else (qc if n == "q" else kc)
                    drain_eng[0 if g % 3 == 0 else 1](dstt[:, g * 1024: (g + 1) * 1024], cp[:])

        def rope(n):
            """x = x*cos + partner(x)*sin on qc/kc (cols b,d,w); q on DVE,
            k on Pool."""
            dstt = qc if n == "q" else kc
            t1 = tmp.tile([128, QCOLS], BF16, tag="tmp")
            t2 = tmp.tile([128, QCOLS], BF16, tag="tmp")
            cos_in = _ap(costab, 0, [[32, D], [1, W], [0, BL]])
            x3 = _ap(dstt, 0, [[128, D], [4, W], [1, BL]])
            t1v = _ap(t1, 0, [[128, D], [4, W], [1, BL]])
            nc.vector.tensor_mul(t1v, x3, cos_in)
            part_in = _ap(dstt, 128, [[256, D // 2], [-128, 2], [4, W], [1, BL]])
            sin_in = _ap(sintab, 0, [[64, D // 2], [32, 2], [1, W], [0, BL]])
            t2v = _ap(t2, 0, [[256, D // 2], [128, 2], [4, W], [1, BL]])
            # the partner*sin product can run on Pool in parallel with DVE
            peng = nc.gpsimd if n == "k" else nc.vector
            peng.tensor_mul(t2v, part_in, sin_in)
            nc.vector.tensor_add(dstt[:], t1[:], t2[:])

        def transpose(n):
            """qc/kc [i, (d,w,b)] -> per-batch qt [64 d, (w,i)]."""
            srct = qc if n == "q" else kc
            for b in range(BL):
                dq = qt[(n, b)]
                for w4 in range(8):
                    tp = tps.tile([64, 512], BF16, tag="tp")
                    for wi in range(4):
                        w = w4 * 4 + wi
                        sv = _ap(srct, w * 4 + b, [[128, D]])
                        nc.tensor.transpose(
                            tp[:, wi * 128: (wi + 1) * 128], sv, ident[:])
                    drain_eng[w4 % 2](
                        dq[:, w4 * 512: (w4 + 1) * 512], tp[:])

        conv("q")
        rope("q")
        conv("k")
        rope("k")
        transpose("q")
        conv("v")
        transpose("k")
        if DEBUG:
            nc.sync.dma_start(out=dbg["qc"][:], in_=qc[:])
            nc.sync.dma_start(out=dbg["kc"][:], in_=kc[:])
            nc.sync.dma_start(out=dbg["vsb"][:], in_=vsb[:])
            nc.sync.dma_start(out=dbg["qt0"][:], in_=qt[("q", 0)][:])
            nc.sync.dma_start(out=dbg["kt0"][:], in_=qt[("k", 0)][:])

        tps.release()
        convps.release()
        tmp.release()
        tpool.release()
        xpool.release()

        # ---------------- attention phase
        simps = tc.alloc_tile_pool(name="simps", bufs=2, space="PSUM")
        avps = tc.alloc_tile_pool(name="avps", bufs=4, space="PSUM")
        ppool = tc.alloc_tile_pool(name="p", bufs=2)
        spool = tc.alloc_tile_pool(name="s", bufs=2)
        opool = tc.alloc_tile_pool(name="o", bufs=2)

        for b in range(BL):
            qtt = qt[("q", b)]
            ktt = qt[("k", b)]
            p = ppool.tile([128, PCOLS], BF16, tag="p")
            ot = opool.tile([128, OTC], BF16, tag="ot")
            # per 4-window group: sim -> exp -> mask -> AV -> drain, so the
            # engines pipeline across groups within a batch
            for t2 in range(8):
                sp = simps.tile([128, 1024], F32, tag="sp")
                ncols_t = 0
                for mi in range(4):
                    m = t2 * 4 + mi
                    ncols = 256 if m < W - 1 else 128
                    nc.tensor.matmul(
                        sp[:, mi * 256: mi * 256 + ncols],
                        ktt[:, m * 128: (m + 1) * 128],
                        qtt[:, m * 128: m * 128 + ncols],
                        start=True, stop=True,
                    )
                    ncols_t += ncols
                nc.scalar.activation(
                    p[:, t2 * 1024: t2 * 1024 + ncols_t],
                    sp[:, :ncols_t],
                    mybir.ActivationFunctionType.Exp,
                )
                if t2 == 0:
                    # key position 0 is pad-masked (torch quirk)
                    nc.vector.memset(p[0:1, 0:256], 0.0)
                # causal mask on own-window halves of this group
                meng = nc.gpsimd if t2 % 2 == 0 else nc.vector
                pview = _ap(p, t2 * 1024, [[256, 4], [1, 128]])
                tri_b = _ap(tri, 0, [[0, 4], [1, 128]])
                meng.tensor_mul(pview, pview, tri_b)
                if t2 == 0:
                    # window-0 query-0: uniform attention over own window
                    nc.vector.memset(p[:, 0:1], 1.0)

                av = avps.tile([128, 4 * 65], F32, tag="av",
                               padded_shape=[128, 512])
                for wi in range(4):
                    w = t2 * 4 + wi
                    slot = wi * 65
                    own = p[:, w * 256: w * 256 + 128]
                    vw = _ap(vsb, w * 4 + b, [[128, 65]])
                    if w == 0:
                        nc.tensor.matmul(av[:, slot: slot + 65], own, vw,
                                         start=True, stop=True)
                    else:
                        nc.tensor.matmul(av[:, slot: slot + 65], own, vw,
                                         start=True, stop=False)
                        prev = p[:, (w - 1) * 256 + 128: w * 256]
                        vprev = _ap(vsb, (w - 1) * 4 + b, [[128, 65]])
                        nc.tensor.matmul(av[:, slot: slot + 65], prev, vprev,
                                         start=False, stop=True)
                if t2 == 0:
                    # +128 phantom pad keys in the w0-q0 denominator
                    nc.vector.tensor_scalar_add(
                        av[0:1, 64:65], av[0:1, 64:65], 128.0)
                sr = spool.tile([128, 4], F32, tag="sr")
                nc.vector.reciprocal(sr[:], _ap(av, 64, [[65, 4]]))
                nc.vector.tensor_mul(
                    _ap(ot, t2 * 256, [[64, 4], [1, 64]]),
                    _ap(av, 0, [[65, 4], [1, 64]]),
                    _ap(sr, 0, [[1, 4], [0, 64]]),
                )
            if DEBUG and b == 0:
                nc.sync.dma_start(out=dbg["p0"][:], in_=p[:])
            nc.scalar.dma_start(
                out=bass.AP(tensor=out, offset=b * 128 * OTC,
                            ap=[[OTC, 128], [1, OTC]]),
                in_=ot[:])

        opool.release()
        spool.release()
        ppool.release()
        avps.release()
        simps.release()
        mid.release()
        const.release()

    _split_multiwaits(nc)
    return nc


_PROG = None


def _get_prog():
    global _PROG
    if _PROG is None:
        _PROG = _build_program()
    return _PROG


def _host_prep(q, k, v, wq, wk, wv):
    """Build per-core input maps (bf16 casts + layout/constant tables)."""
    jj = np.arange(WS)[:, None]   # jin
    ii = np.arange(WS)[None, :]   # jout
    hid = jj - ii + (WS - 1)      # hi: w[jin-jout+127] for jin <= jout
    lod = jj - ii - 1             # lo: w[jin-jout-1]   for jin >  jout
    him = (hid >= 0) & (hid < WS)
    lom = (lod >= 0) & (lod < WS)
    hidc = np.clip(hid, 0, WS - 1)
    lodc = np.clip(lod, 0, WS - 1)

    def toep(w, scale=1.0):
        wd = np.asarray(w, np.float32).reshape(D, WS) * scale
        # t[jin, d, {hi,lo}, jout]
        t = np.zeros((WS, D, 2, WS), np.float32)
        hi = np.where(him, wd[:, hidc], 0.0)          # [D, jin, jout]
        lo = np.where(lom, wd[:, lodc], 0.0)
        t[:, :, 0, :] = hi.transpose(1, 0, 2)
        t[:, :, 1, :] = lo.transpose(1, 0, 2)
        return np.ascontiguousarray(t.reshape(WS, TCOLS)).astype(NPBF)

    tq_np = toep(wq, SCALE)
    tk_np = toep(wk)
    tv_np = toep(wv)

    theta = 1.0 / ROPE_BASE ** (np.arange(0, D, 2, dtype=np.float32) / D)
    pm = np.arange(N, dtype=np.float32)[:, None] * theta[None, :]
    cos = np.repeat(np.cos(pm), 2, axis=-1)  # [n, d]
    sin = np.repeat(np.sin(pm), 2, axis=-1)
    sgn = np.where(np.arange(D) % 2 == 0, -1.0, 1.0).astype(np.float32)
    # [i, (d, w)] layout
    cosb_np = np.ascontiguousarray(
        cos.reshape(W, WS, D).transpose(1, 2, 0).reshape(WS, D * W)
    ).astype(NPBF)
    sinb_np = np.ascontiguousarray(
        (sin * sgn[None, :]).reshape(W, WS, D).transpose(1, 2, 0).reshape(WS, D * W)
    ).astype(NPBF)

    def xtile(x, sl):
        # [BL, N, D] f32 -> [128 i, (w'=33, b, d)] bf16 with w'=0 zeroed
        xb = np.asarray(x[sl], np.float32).astype(NPBF)
        buf = np.zeros((WS, NWP, BL, D), NPBF)
        buf[:, 1:] = xb.reshape(BL, W, WS, D).transpose(2, 1, 0, 3)
        return np.ascontiguousarray(buf.reshape(WS, XCOLS))

    in_maps = []
    for c in range(NCORES):
        sl = slice(c * BL, (c + 1) * BL)
        in_maps.append({
            "xq": xtile(q, sl), "xk": xtile(k, sl), "xv": xtile(v, sl),
            "tq": tq_np, "tk": tk_np, "tv": tv_np,
            "cosb": cosb_np, "sinb": sinb_np,
        })
    return in_maps


def _install_ntff_hook():
    """Provide antenv.axon_hooks with a ctypes NTFF profile hook (the slim
    container lacks it); enables trace=True under axon."""
    import sys as _sys
    import types
    import ctypes
    import contextlib

    try:
        from antenv.axon_hooks import get_axon_ntff_profile_hook  # noqa: F401
        return
    except ImportError:
        pass
    so_path = "/opt/axon/libaxon_pjrt.so"
    try:
        lib = ctypes.CDLL(so_path)
    except OSError:
        return
    if not hasattr(lib, "axon_start_nrt_profile"):
        return
    lib.axon_start_nrt_profile.argtypes = [
        ctypes.POINTER(ctypes.c_int64), ctypes.c_size_t]
    lib.axon_start_nrt_profile.restype = ctypes.c_int64
    lib.axon_stop_nrt_profile.argtypes = [ctypes.c_char_p]
    lib.axon_stop_nrt_profile.restype = ctypes.c_int64

    @contextlib.contextmanager
    def _hook(output_dir, device_ids):
        import jax
        jax.devices()
        if device_ids:
            ids = (ctypes.c_int64 * len(device_ids))(*device_ids)
            rc = lib.axon_start_nrt_profile(ids, len(device_ids))
        else:
            rc = lib.axon_start_nrt_profile(None, 0)
        if rc != 0:
            raise RuntimeError(f"axon_start_nrt_profile rc={rc}")
        try:
            yield
        finally:
            n = lib.axon_stop_nrt_profile(str(output_dir).encode())
            print(f"profile: {n} file(s) written to {output_dir}")

    import antenv

    mod = types.ModuleType("antenv.axon_hooks")
    _state = {"hook": _hook}
    mod.set_axon_ntff_profile_hook = lambda h: _state.__setitem__("hook", h)
    mod.get_axon_ntff_profile_hook = lambda: _state["hook"]
    _sys.modules["antenv.axon_hooks"] = mod
    antenv.axon_hooks = mod


def run(q, k, v, wq, wk, wv, trace=False):
    nc = _get_prog()
    in_maps = _host_prep(q, k, v, wq, wk, wv)
    if trace:
        _install_ntff_hook()
    res = run_bass_kernel_spmd(nc, in_maps, core_ids=list(range(NCORES)),
                               trace=trace)
    outs = []
    for c in range(NCORES):
        ob = np.asarray(res.results[c]["out"], np.float32)
        # [BL*128, (w, d)] -> [BL, N, D]
        outs.append(ob.reshape(BL, WS, W, D).transpose(0, 2, 1, 3)
                    .reshape(BL, N, D))
    outp = np.ascontiguousarray(np.concatenate(outs, axis=0))
    return outp, res


def kernel(q, k, v, wq, wk, wv):
    outp, _ = run(q, k, v, wq, wk, wv)
    return outp


# revision 26
# speedup vs baseline: 1.0053x; 1.0053x over previous
"""Trainium2 Bass kernel for nn_LocalAttention (depthwise causal conv + RoPE +
windowed local attention), data-parallel over the batch dim on 8 NeuronCores.

Self-contained: hardcodes shapes B=32, N=4096, D=64, WS=128 and the sharding
(4 batches per core). Host-side prep is limited to dtype casts and layout
transforms (Toeplitz band tables from the depthwise conv weights, RoPE cos/sin
tables, tiling of x into the SBUF partition layout); all FLOPs over the
activations run on device.

Layout/scheduling notes (349us baseline -> ~226us):
- All DRAM inputs are pre-tiled on host so every DMA is [128 partitions x
  big-contiguous-run]: ~2.5K descriptors total instead of ~115K, and the
  load order (xq + toeplitz chunks on the sync queue, xk/xv/tables on the
  scalar queue) lets conv-q start at ~10us instead of ~36us.
- qc/kc/vsb use (d, w, b)-style column layouts so every conv PSUM drain is
  a contiguous [128, 1024] copy (strided scatters cost 4x on DVE/ACT).
- The softmax-denominator ones-column is folded into V (65-wide AV
  matmuls); AV PSUM groups are 4 windows padded to one 2KB bank (a
  [128, 520] f32 tile crosses a bank boundary mid-matmul and corrupts).
- Attention runs per-4-window-group (sim -> exp -> causal-mask -> AV ->
  scaled drain) so ACT/DVE/Pool pipeline behind the PE within a batch.
- Activation engine does exp + some drains; causal mask on Pool/DVE.
- Output is written bf16 in on-chip layout and fixed up on host.
- Remaining wall time is PE-bound: this toolchain serializes a ~M/1.2GHz
  LDWEIGHTS before every matmul (enable-ldw-opt=false, incompatible with
  the explicit InstLdweights bass emits), so each matmul costs about
  (M + N) * 0.83ns and the conv/transpose/sim/AV total is ~244K col-units.
  A HAM warm-up burst was tested: the clock gate does flip to 2.4GHz but
  per-matmul time is unchanged (LDW path + issue overhead bound), so it
  was removed.
"""

import sys

sys.path.insert(0, "/opt/trn_rl_repo")

import ml_dtypes
import numpy as np

import concourse.bass as bass
import concourse.mybir as mybir
import concourse.tile as tile
from concourse.bass_utils import run_bass_kernel_spmd
from concourse.masks import make_identity


BF16 = mybir.dt.bfloat16
F32 = mybir.dt.float32
NPBF = ml_dtypes.bfloat16

B, N, D, WS = 32, 4096, 64, 128
W = N // WS              # 32 windows
NCORES = 8
BL = B // NCORES         # 4 batches per core
NWP = W + 1              # 33 window slots (slot 0 = zero pad = "window -1")
SCALE = D ** -0.5
XCOLS = NWP * BL * D     # 8448   x cols (w', b, d)
TCOLS = D * 2 * WS       # 16384  toeplitz cols (d, {hi,lo}, jout)
QCOLS = BL * D * W       # 8192   qc/kc cols (b, d, w)
VCOLS = BL * W * 65      # 8320   vsb cols (b, w, d+ones)
PCOLS = W * 2 * WS       # 8192   p cols (m, 256)
OTC = W * D              # 2048   out cols (w, d)
ROPE_BASE = 10000.0
DEBUG = False


def _split_multiwaits(nc, max_waits=1):
    """walrus in this env rejects >1 sem wait per instruction; split extras
    into standalone NoOp waits inserted just before, on the same engine."""
    n_fixed = 0
    for fn in nc.m.functions:
        for bb in fn.blocks:
            insts = bb.instructions
            new_list = []
            changed = False
            for inst in insts:
                si = inst.sync_info
                if si is not None and si.on_wait and len(si.on_wait) > max_waits:
                    waits = list(si.on_wait)
                    for w in waits[:-max_waits]:
                        nop = mybir.InstNoOp(
                            name=f"{inst.name}-xw{n_fixed}",
                            engine=inst.engine,
                            ins=[],
                            outs=[],
                            sync_info=mybir.SyncInfo(on_wait=[w], on_update=[]),
                        )
                        new_list.append(nop)
                        n_fixed += 1
                    si.on_wait = waits[-max_waits:]
                    changed = True
                new_list.append(inst)
            if changed:
                bb.instructions = new_list
    return n_fixed


def _ap(t, offset, dims):
    """AP over tile/dram tensor t: partition dim kept, free dims replaced."""
    return bass.AP(tensor=t.tensor, offset=t.offset + offset, ap=[t.ap[0]] + dims)


def _build_program():
    nc = bass.Bass()
    xd = {n: nc.dram_tensor(f"x{n}", [128, XCOLS], BF16, kind="ExternalInput")
          for n in ("q", "k", "v")}
    td = {n: nc.dram_tensor(f"t{n}", [128, TCOLS], BF16, kind="ExternalInput")
          for n in ("q", "k", "v")}
    cosb = nc.dram_tensor("cosb", [128, D * W], BF16, kind="ExternalInput")
    sinb = nc.dram_tensor("sinb", [128, D * W], BF16, kind="ExternalInput")
    out = nc.dram_tensor("out", [BL * 128, OTC], BF16, kind="ExternalOutput")
    dbg = {}
    if DEBUG:
        dbg["qc"] = nc.dram_tensor("dbg_qc", [128, QCOLS], BF16, kind="ExternalOutput")
        dbg["kc"] = nc.dram_tensor("dbg_kc", [128, QCOLS], BF16, kind="ExternalOutput")
        dbg["vsb"] = nc.dram_tensor("dbg_vsb", [128, VCOLS], BF16, kind="ExternalOutput")
        dbg["p0"] = nc.dram_tensor("dbg_p0", [128, PCOLS], BF16, kind="ExternalOutput")
        dbg["qt0"] = nc.dram_tensor("dbg_qt0", [128, W * WS], BF16, kind="ExternalOutput")
        dbg["kt0"] = nc.dram_tensor("dbg_kt0", [128, W * WS], BF16, kind="ExternalOutput")

    with tile.TileContext(nc) as tc:
        # ---------------- persistent pools
        const = tc.alloc_tile_pool(name="const", bufs=1)
        mid = tc.alloc_tile_pool(name="mid", bufs=1)

        ident = const.tile([128, 128], BF16, tag="ident")
        make_identity(nc, ident)
        tri = const.tile([128, 128], BF16, tag="tri")  # tri[j,i]=1 iff i>=j
        nc.vector.memset(tri[:], 1.0)
        nc.gpsimd.affine_select(
            out=tri[:], in_=tri[:], compare_op=mybir.AluOpType.is_ge,
            fill=0.0, base=0, channel_multiplier=-1, pattern=[[1, 128]],
        )
        costab = const.tile([128, D * W], BF16, tag="cos")
        nc.scalar.dma_start(out=costab[:], in_=cosb[:])
        sintab = const.tile([128, D * W], BF16, tag="sin")
        nc.scalar.dma_start(out=sintab[:], in_=sinb[:])

        qc = mid.tile([128, QCOLS], BF16, tag="qc")
        kc = mid.tile([128, QCOLS], BF16, tag="kc")
        vsb = mid.tile([128, VCOLS], BF16, tag="vsb")
        # ones block (channel 64) for the folded softmax denominator
        nc.vector.memset(vsb[:, 64 * 128: 65 * 128], 1.0)
        qt = {}
        for _n in ("q", "k"):
            for _b in range(BL):
                qt[(_n, _b)] = mid.tile([64, W * WS], BF16,
                                        tag=f"{_n}t{_b}", name=f"{_n}t{_b}")

        # ---------------- conv + rope + transpose phase
        xpool = tc.alloc_tile_pool(name="x", bufs=2)
        tpool = tc.alloc_tile_pool(name="toep", bufs=2)
        tmp = tc.alloc_tile_pool(name="tmp", bufs=2)
        convps = tc.alloc_tile_pool(name="convps", bufs=3, space="PSUM")
        tps = tc.alloc_tile_pool(name="tps", bufs=2, space="PSUM")

        # input loads: one big contiguous DMA per tensor, toeplitz in 4 chunks
        xt = {}
        for n in ("q", "k", "v"):
            t = xpool.tile([128, XCOLS], BF16, tag="x", name=f"x{n}")
            # xq on the sync queue ahead of the tq chunks; xk/xv go down the
            # scalar queue so they don't delay conv-q's toeplitz chunks
            eng = nc.sync if n == "q" else nc.scalar
            eng.dma_start(out=t[:], in_=xd[n][:])
            xt[n] = t

        drain_eng = [nc.vector.tensor_copy, nc.scalar.copy]


        def conv(n):
            """Depthwise causal conv for tensor n via per-channel Toeplitz
            matmuls; drains into qc/kc (cols b,d,w) or vsb (cols b,w,d)."""
            x = xt[n]
            for c in range(4):           # 16-channel toeplitz chunks
                tt = tpool.tile([128, 16 * 256], BF16, tag="toep")
                nc.sync.dma_start(
                    out=tt[:],
                    in_=bass.AP(tensor=td[n], offset=c * 4096,
                                ap=[[TCOLS, 128], [1, 4096]]))
                for g2 in range(2):      # psum groups of 8 channels
                    g = c * 2 + g2
                    cp = convps.tile([128, 8 * 128], F32, tag="cp")
                    for dd in range(8):
                        d = g * 8 + dd
                        dloc = d % 16
                        hi = tt[:, dloc * 256: dloc * 256 + 128]
                        lo = tt[:, dloc * 256 + 128: dloc * 256 + 256]
                        # moving: x cols (w', b) at channel d
                        rhs_hi = _ap(x, BL * D + d, [[BL * D, W], [D, BL]])
                        rhs_lo = _ap(x, d, [[BL * D, W], [D, BL]])
                        ps = cp[:, dd * 128: (dd + 1) * 128]
                        nc.tensor.matmul(ps, hi, rhs_hi, start=True, stop=False)
                        nc.tensor.matmul(ps, lo, rhs_lo, start=False, stop=True)
                    # drain; cp is [128 jout, (8 d, 32 w, 4 b)] and the
                    # qc/kc/vsb layouts are (d, w, b) so the copy is contiguous
                    dstt = vsb if n == "v" else (qc if n == "q" else kc)
                    drain_eng[0 if g % 3 == 0 else 1](dstt[:, g * 1024: (g + 1) * 1024], cp[:])

        def rope(n):
            """x = x*cos + partner(x)*sin on qc/kc (cols b,d,w); q on DVE,
            k on Pool."""
            dstt = qc if n == "q" else kc
            t1 = tmp.tile([128, QCOLS], BF16, tag="tmp")
            t2 = tmp.tile([128, QCOLS], BF16, tag="tmp")
            cos_in = _ap(costab, 0, [[32, D], [1, W], [0, BL]])
            x3 = _ap(dstt, 0, [[128, D], [4, W], [1, BL]])
            t1v = _ap(t1, 0, [[128, D], [4, W], [1, BL]])
            nc.vector.tensor_mul(t1v, x3, cos_in)
            part_in = _ap(dstt, 128, [[256, D // 2], [-128, 2], [4, W], [1, BL]])
            sin_in = _ap(sintab, 0, [[64, D // 2], [32, 2], [1, W], [0, BL]])
            t2v = _ap(t2, 0, [[256, D // 2], [128, 2], [4, W], [1, BL]])
            # the partner*sin product can run on Pool in parallel with DVE
            peng = nc.gpsimd if n == "k" else nc.vector
            peng.tensor_mul(t2v, part_in, sin_in)
            nc.vector.tensor_add(dstt[:], t1[:], t2[:])

        def transpose(n):
            """qc/kc [i, (d,w,b)] -> per-batch qt [64 d, (w,i)]."""
            srct = qc if n == "q" else kc
            for b in range(BL):
                dq = qt[(n, b)]
                for w4 in range(8):
                    tp = tps.tile([64, 512], BF16, tag="tp")
                    for wi in range(4):
                        w = w4 * 4 + wi
                        sv = _ap(srct, w * 4 + b, [[128, D]])
                        nc.tensor.transpose(
                            tp[:, wi * 128: (wi + 1) * 128], sv, ident[:])
                    drain_eng[w4 % 2](
                        dq[:, w4 * 512: (w4 + 1) * 512], tp[:])

        conv("q")
        rope("q")
        conv("k")
        rope("k")
        transpose("q")
        conv("v")
        transpose("k")
        if DEBUG:
            nc.sync.dma_start(out=dbg["qc"][:], in_=qc[:])
            nc.sync.dma_start(out=dbg["kc"][:], in_=kc[:])
            nc.sync.dma_start(out=dbg["vsb"][:], in_=vsb[:])
            nc.sync.dma_start(out=dbg["qt0"][:], in_=qt[("q", 0)][:])
            nc.sync.dma_start(out=dbg["kt0"][:], in_=qt[("k", 0)][:])

        tps.release()
        convps.release()
        tmp.release()
        tpool.release()
        xpool.release()

        # ---------------- attention phase
        simps = tc.alloc_tile_pool(name="simps", bufs=2, space="PSUM")
        avps = tc.alloc_tile_pool(name="avps", bufs=4, space="PSUM")
        ppool = tc.alloc_tile_pool(name="p", bufs=2)
        spool = tc.alloc_tile_pool(name="s", bufs=2)
        opool = tc.alloc_tile_pool(name="o", bufs=2)

        for b in range(BL):
            qtt = qt[("q", b)]
            ktt = qt[("k", b)]
            p = ppool.tile([128, PCOLS], BF16, tag="p")
            ot = opool.tile([128, OTC], BF16, tag="ot")
            # per 4-window group: sim -> exp -> mask -> AV -> drain, so the
            # engines pipeline across groups within a batch
            for t2 in range(8):
                sp = simps.tile([128, 1024], F32, tag="sp")
                ncols_t = 0
                for mi in range(4):
                    m = t2 * 4 + mi
                    ncols = 256 if m < W - 1 else 128
                    nc.tensor.matmul(
                        sp[:, mi * 256: mi * 256 + ncols],
                        ktt[:, m * 128: (m + 1) * 128],
                        qtt[:, m * 128: m * 128 + ncols],
                        start=True, stop=True,
                    )
                    ncols_t += ncols
                nc.scalar.activation(
                    p[:, t2 * 1024: t2 * 1024 + ncols_t],
                    sp[:, :ncols_t],
                    mybir.ActivationFunctionType.Exp,
                )
                if t2 == 0:
                    # key position 0 is pad-masked (torch quirk)
                    nc.vector.memset(p[0:1, 0:256], 0.0)
                # causal mask on own-window halves of this group
                meng = nc.gpsimd if t2 % 2 == 0 else nc.vector
                pview = _ap(p, t2 * 1024, [[256, 4], [1, 128]])
                tri_b = _ap(tri, 0, [[0, 4], [1, 128]])
                meng.tensor_mul(pview, pview, tri_b)
                if t2 == 0:
                    # window-0 query-0: uniform attention over own window
                    nc.vector.memset(p[:, 0:1], 1.0)

                av = avps.tile([128, 4 * 65], F32, tag="av",
                               padded_shape=[128, 512])
                for wi in range(4):
                    w = t2 * 4 + wi
                    slot = wi * 65
                    own = p[:, w * 256: w * 256 + 128]
                    vw = _ap(vsb, w * 4 + b, [[128, 65]])
                    if w == 0:
                        nc.tensor.matmul(av[:, slot: slot + 65], own, vw,
                                         start=True, stop=True)
                    else:
                        nc.tensor.matmul(av[:, slot: slot + 65], own, vw,
                                         start=True, stop=False)
                        prev = p[:, (w - 1) * 256 + 128: w * 256]
                        vprev = _ap(vsb, (w - 1) * 4 + b, [[128, 65]])
                        nc.tensor.matmul(av[:, slot: slot + 65], prev, vprev,
                                         start=False, stop=True)
                if t2 == 0:
                    # +128 phantom pad keys in the w0-q0 denominator
                    nc.vector.tensor_scalar_add(
                        av[0:1, 64:65], av[0:1, 64:65], 128.0)
                sr = spool.tile([128, 4], F32, tag="sr")
                nc.vector.reciprocal(sr[:], _ap(av, 64, [[65, 4]]))
                nc.vector.tensor_mul(
                    _ap(ot, t2 * 256, [[64, 4], [1, 64]]),
                    _ap(av, 0, [[65, 4], [1, 64]]),
                    _ap(sr, 0, [[1, 4], [0, 64]]),
                )
            if DEBUG and b == 0:
                nc.sync.dma_start(out=dbg["p0"][:], in_=p[:])
            nc.scalar.dma_start(
                out=bass.AP(tensor=out, offset=b * 128 * OTC,
                            ap=[[OTC, 128], [1, OTC]]),
                in_=ot[:])

        opool.release()
        spool.release()
        ppool.release()
        avps.release()
        simps.release()
        mid.release()
        const.release()

    _split_multiwaits(nc)
    return nc


_PROG = None


def _get_prog():
    global _PROG
    if _PROG is None:
        _PROG = _build_program()
    return _PROG


def _host_prep(q, k, v, wq, wk, wv):
    """Build per-core input maps (bf16 casts + layout/constant tables)."""
    jj = np.arange(WS)[:, None]   # jin
    ii = np.arange(WS)[None, :]   # jout
    hid = jj - ii + (WS - 1)      # hi: w[jin-jout+127] for jin <= jout
    lod = jj - ii - 1             # lo: w[jin-jout-1]   for jin >  jout
    him = (hid >= 0) & (hid < WS)
    lom = (lod >= 0) & (lod < WS)
    hidc = np.clip(hid, 0, WS - 1)
    lodc = np.clip(lod, 0, WS - 1)

    def toep(w, scale=1.0):
        wd = np.asarray(w, np.float32).reshape(D, WS) * scale
        # t[jin, d, {hi,lo}, jout]
        t = np.zeros((WS, D, 2, WS), np.float32)
        hi = np.where(him, wd[:, hidc], 0.0)          # [D, jin, jout]
        lo = np.where(lom, wd[:, lodc], 0.0)
        t[:, :, 0, :] = hi.transpose(1, 0, 2)
        t[:, :, 1, :] = lo.transpose(1, 0, 2)
        return np.ascontiguousarray(t.reshape(WS, TCOLS)).astype(NPBF)

    tq_np = toep(wq, SCALE)
    tk_np = toep(wk)
    tv_np = toep(wv)

    theta = 1.0 / ROPE_BASE ** (np.arange(0, D, 2, dtype=np.float32) / D)
    pm = np.arange(N, dtype=np.float32)[:, None] * theta[None, :]
    cos = np.repeat(np.cos(pm), 2, axis=-1)  # [n, d]
    sin = np.repeat(np.sin(pm), 2, axis=-1)
    sgn = np.where(np.arange(D) % 2 == 0, -1.0, 1.0).astype(np.float32)
    # [i, (d, w)] layout
    cosb_np = np.ascontiguousarray(
        cos.reshape(W, WS, D).transpose(1, 2, 0).reshape(WS, D * W)
    ).astype(NPBF)
    sinb_np = np.ascontiguousarray(
        (sin * sgn[None, :]).reshape(W, WS, D).transpose(1, 2, 0).reshape(WS, D * W)
    ).astype(NPBF)

    def xtile(x, sl):
        # [BL, N, D] f32 -> [128 i, (w'=33, b, d)] bf16 with w'=0 zeroed
        xb = np.asarray(x[sl], np.float32).astype(NPBF)
        buf = np.zeros((WS, NWP, BL, D), NPBF)
        buf[:, 1:] = xb.reshape(BL, W, WS, D).transpose(2, 1, 0, 3)
        return np.ascontiguousarray(buf.reshape(WS, XCOLS))

    in_maps = []
    for c in range(NCORES):
        sl = slice(c * BL, (c + 1) * BL)
        in_maps.append({
            "xq": xtile(q, sl), "xk": xtile(k, sl), "xv": xtile(v, sl),
            "tq": tq_np, "tk": tk_np, "tv": tv_np,
            "cosb": cosb_np, "sinb": sinb_np,
        })
    return in_maps


def _install_ntff_hook():
    """Provide antenv.axon_hooks with a ctypes NTFF profile hook (the slim
    container lacks it); enables trace=True under axon."""
    import sys as _sys
    import types
    import ctypes
    import contextlib

    try:
        from antenv.axon_hooks import get_axon_ntff_profile_hook  # noqa: F401
        return
    except ImportError:
        pass
    so_path = "/opt/axon/libaxon_pjrt.so"
    try:
        lib = ctypes.CDLL(so_path)
    except OSError:
        return
    if not hasattr(lib, "axon_start_nrt_profile"):
        return
    lib.axon_start_nrt_profile.argtypes = [
        ctypes.POINTER(ctypes.c_int64), ctypes.c_size_t]
    lib.axon_start_nrt_profile.restype = ctypes.c_int64
    lib.axon_stop_nrt_profile.argtypes = [ctypes.c_char_p]
    lib.axon_stop_nrt_profile.restype = ctypes.c_int64

    @contextlib.contextmanager
    def _hook(output_dir, device_ids):
        import jax
        jax.devices()
        if device_ids:
            ids = (ctypes.c_int64 * len(device_ids))(*device_ids)
            rc = lib.axon_start_nrt_profile(ids, len(device_ids))
        else:
            rc = lib.axon_start_nrt_profile(None, 0)
        if rc != 0:
            raise RuntimeError(f"axon_start_nrt_profile rc={rc}")
        try:
            yield
        finally:
            n = lib.axon_stop_nrt_profile(str(output_dir).encode())
            print(f"profile: {n} file(s) written to {output_dir}")

    import antenv

    mod = types.ModuleType("antenv.axon_hooks")
    _state = {"hook": _hook}
    mod.set_axon_ntff_profile_hook = lambda h: _state.__setitem__("hook", h)
    mod.get_axon_ntff_profile_hook = lambda: _state["hook"]
    _sys.modules["antenv.axon_hooks"] = mod
    antenv.axon_hooks = mod


def run(q, k, v, wq, wk, wv, trace=False):
    nc = _get_prog()
    in_maps = _host_prep(q, k, v, wq, wk, wv)
    if trace:
        _install_ntff_hook()
    res = run_bass_kernel_spmd(nc, in_maps, core_ids=list(range(NCORES)),
                               trace=trace)
    outs = []
    for c in range(NCORES):
        ob = np.asarray(res.results[c]["out"], np.float32)
        # [BL*128, (w, d)] -> [BL, N, D]
        outs.append(ob.reshape(BL, WS, W, D).transpose(0, 2, 1, 3)
                    .reshape(BL, N, D))
    outp = np.ascontiguousarray(np.concatenate(outs, axis=0))
    return outp, res


def kernel(q, k, v, wq, wk, wv):
    outp, _ = run(q, k, v, wq, wk, wv)
    return outp


# revision 27
# speedup vs baseline: 1.0291x; 1.0237x over previous
"""Trainium2 Bass kernel for nn_LocalAttention (depthwise causal conv + RoPE +
windowed local attention), data-parallel over the batch dim on 8 NeuronCores.

Self-contained: hardcodes shapes B=32, N=4096, D=64, WS=128 and the sharding
(4 batches per core). Host-side prep is limited to dtype casts and layout
transforms (Toeplitz band tables from the depthwise conv weights, RoPE cos/sin
tables, tiling of x into the SBUF partition layout); all FLOPs over the
activations run on device.

Layout/scheduling notes (349us baseline -> ~226us):
- All DRAM inputs are pre-tiled on host so every DMA is [128 partitions x
  big-contiguous-run]: ~2.5K descriptors total instead of ~115K, and the
  load order (xq + toeplitz chunks on the sync queue, xk/xv/tables on the
  scalar queue) lets conv-q start at ~10us instead of ~36us.
- qc/kc/vsb use (d, w, b)-style column layouts so every conv PSUM drain is
  a contiguous [128, 1024] copy (strided scatters cost 4x on DVE/ACT).
- The softmax-denominator ones-column is folded into V (65-wide AV
  matmuls); AV PSUM groups are 4 windows padded to one 2KB bank (a
  [128, 520] f32 tile crosses a bank boundary mid-matmul and corrupts).
- Attention runs per-4-window-group (sim -> exp -> causal-mask -> AV ->
  scaled drain) so ACT/DVE/Pool pipeline behind the PE within a batch.
- Activation engine does exp + some drains; causal mask on Pool/DVE.
- Output is written bf16 in on-chip layout and fixed up on host.
- Remaining wall time is PE-bound: this toolchain serializes a ~M/1.2GHz
  LDWEIGHTS before every matmul (enable-ldw-opt=false, incompatible with
  the explicit InstLdweights bass emits), so each matmul costs about
  (M + N) * 0.83ns and the conv/transpose/sim/AV total is ~244K col-units.
  A HAM warm-up burst was tested: the clock gate does flip to 2.4GHz but
  per-matmul time is unchanged (LDW path + issue overhead bound), so it
  was removed.
"""

import sys

sys.path.insert(0, "/opt/trn_rl_repo")

import ml_dtypes
import numpy as np

import concourse.bass as bass
import concourse.mybir as mybir
import concourse.tile as tile
from concourse.bass_utils import run_bass_kernel_spmd
from concourse.masks import make_identity


BF16 = mybir.dt.bfloat16
F32 = mybir.dt.float32
NPBF = ml_dtypes.bfloat16

B, N, D, WS = 32, 4096, 64, 128
W = N // WS              # 32 windows
NCORES = 8
BL = B // NCORES         # 4 batches per core
NWP = W + 1              # 33 window slots (slot 0 = zero pad = "window -1")
SCALE = D ** -0.5
XCOLS = NWP * BL * D     # 8448   x cols (w', b, d)
TCOLS = D * 2 * WS       # 16384  toeplitz cols (d, {hi,lo}, jout)
QCOLS = BL * D * W       # 8192   qc/kc cols (b, d, w)
VCOLS = BL * W * 65      # 8320   vsb cols (b, w, d+ones)
PCOLS = W * 2 * WS       # 8192   p cols (m, 256)
OTC = W * D              # 2048   out cols (w, d)
ROPE_BASE = 10000.0
DEBUG = False


def _split_multiwaits(nc, max_waits=1):
    """walrus in this env rejects >1 sem wait per instruction; split extras
    into standalone NoOp waits inserted just before, on the same engine."""
    n_fixed = 0
    for fn in nc.m.functions:
        for bb in fn.blocks:
            insts = bb.instructions
            new_list = []
            changed = False
            for inst in insts:
                si = inst.sync_info
                if si is not None and si.on_wait and len(si.on_wait) > max_waits:
                    waits = list(si.on_wait)
                    for w in waits[:-max_waits]:
                        nop = mybir.InstNoOp(
                            name=f"{inst.name}-xw{n_fixed}",
                            engine=inst.engine,
                            ins=[],
                            outs=[],
                            sync_info=mybir.SyncInfo(on_wait=[w], on_update=[]),
                        )
                        new_list.append(nop)
                        n_fixed += 1
                    si.on_wait = waits[-max_waits:]
                    changed = True
                new_list.append(inst)
            if changed:
                bb.instructions = new_list
    return n_fixed


def _ap(t, offset, dims):
    """AP over tile/dram tensor t: partition dim kept, free dims replaced."""
    return bass.AP(tensor=t.tensor, offset=t.offset + offset, ap=[t.ap[0]] + dims)


def _build_program():
    nc = bass.Bass()
    xd = {n: nc.dram_tensor(f"x{n}", [128, XCOLS], BF16, kind="ExternalInput")
          for n in ("q", "k", "v")}
    td = {n: nc.dram_tensor(f"t{n}", [128, TCOLS], BF16, kind="ExternalInput")
          for n in ("q", "k", "v")}
    cosb = nc.dram_tensor("cosb", [128, D * W], BF16, kind="ExternalInput")
    sinb = nc.dram_tensor("sinb", [128, D * W], BF16, kind="ExternalInput")
    out = nc.dram_tensor("out", [BL * 128, OTC], BF16, kind="ExternalOutput")
    dbg = {}
    if DEBUG:
        dbg["qc"] = nc.dram_tensor("dbg_qc", [128, QCOLS], BF16, kind="ExternalOutput")
        dbg["kc"] = nc.dram_tensor("dbg_kc", [128, QCOLS], BF16, kind="ExternalOutput")
        dbg["vsb"] = nc.dram_tensor("dbg_vsb", [128, VCOLS], BF16, kind="ExternalOutput")
        dbg["p0"] = nc.dram_tensor("dbg_p0", [128, PCOLS], BF16, kind="ExternalOutput")
        dbg["qt0"] = nc.dram_tensor("dbg_qt0", [128, W * WS], BF16, kind="ExternalOutput")
        dbg["kt0"] = nc.dram_tensor("dbg_kt0", [128, W * WS], BF16, kind="ExternalOutput")

    with tile.TileContext(nc) as tc:
        # ---------------- persistent pools
        const = tc.alloc_tile_pool(name="const", bufs=1)
        mid = tc.alloc_tile_pool(name="mid", bufs=1)

        ident = const.tile([128, 128], BF16, tag="ident")
        make_identity(nc, ident)
        tri = const.tile([128, 128], BF16, tag="tri")  # tri[j,i]=1 iff i>=j
        nc.vector.memset(tri[:], 1.0)
        nc.gpsimd.affine_select(
            out=tri[:], in_=tri[:], compare_op=mybir.AluOpType.is_ge,
            fill=0.0, base=0, channel_multiplier=-1, pattern=[[1, 128]],
        )
        costab = const.tile([128, D * W], BF16, tag="cos")
        nc.scalar.dma_start(out=costab[:], in_=cosb[:])
        sintab = const.tile([128, D * W], BF16, tag="sin")
        nc.scalar.dma_start(out=sintab[:], in_=sinb[:])

        qc = mid.tile([128, QCOLS], BF16, tag="qc")
        kc = mid.tile([128, QCOLS], BF16, tag="kc")
        vsb = mid.tile([128, VCOLS], BF16, tag="vsb")
        # ones block (channel 64) for the folded softmax denominator
        nc.vector.memset(vsb[:, 64 * 128: 65 * 128], 1.0)
        qt = {}
        for _n in ("q", "k"):
            for _b in range(BL):
                qt[(_n, _b)] = mid.tile([64, W * WS], BF16,
                                        tag=f"{_n}t{_b}", name=f"{_n}t{_b}")

        # ---------------- conv + rope + transpose phase
        xpool = tc.alloc_tile_pool(name="x", bufs=2)
        tpool = tc.alloc_tile_pool(name="toep", bufs=2)
        tmp = tc.alloc_tile_pool(name="tmp", bufs=2)
        convps = tc.alloc_tile_pool(name="convps", bufs=3, space="PSUM")
        tps = tc.alloc_tile_pool(name="tps", bufs=2, space="PSUM")

        # input loads: one big contiguous DMA per tensor, toeplitz in 4
        # chunks; each tensor's x load is enqueued just before its conv so
        # earlier-needed transfers get the full DMA bandwidth
        xt = {}

        def load_x(n):
            t = xpool.tile([128, XCOLS], BF16, tag="x", name=f"x{n}")
            nc.sync.dma_start(out=t[:], in_=xd[n][:])
            xt[n] = t

        load_x("q")

        drain_eng = [nc.vector.tensor_copy, nc.scalar.copy]


        def conv(n):
            """Depthwise causal conv for tensor n via per-channel Toeplitz
            matmuls; drains into qc/kc (cols b,d,w) or vsb (cols b,w,d)."""
            if n not in xt:
                load_x(n)
            x = xt[n]
            for c in range(4):           # 16-channel toeplitz chunks
                tt = tpool.tile([128, 16 * 256], BF16, tag="toep")
                nc.sync.dma_start(
                    out=tt[:],
                    in_=bass.AP(tensor=td[n], offset=c * 4096,
                                ap=[[TCOLS, 128], [1, 4096]]))
                for g2 in range(2):      # psum groups of 8 channels
                    g = c * 2 + g2
                    cp = convps.tile([128, 8 * 128], F32, tag="cp")
                    for dd in range(8):
                        d = g * 8 + dd
                        dloc = d % 16
                        hi = tt[:, dloc * 256: dloc * 256 + 128]
                        lo = tt[:, dloc * 256 + 128: dloc * 256 + 256]
                        # moving: x cols (w', b) at channel d
                        rhs_hi = _ap(x, BL * D + d, [[BL * D, W], [D, BL]])
                        rhs_lo = _ap(x, d, [[BL * D, W], [D, BL]])
                        ps = cp[:, dd * 128: (dd + 1) * 128]
                        nc.tensor.matmul(ps, hi, rhs_hi, start=True, stop=False)
                        nc.tensor.matmul(ps, lo, rhs_lo, start=False, stop=True)
                    # drain; cp is [128 jout, (8 d, 32 w, 4 b)] and the
                    # qc/kc/vsb layouts are (d, w, b) so the copy is contiguous
                    dstt = vsb if n == "v" else (qc if n == "q" else kc)
                    drain_eng[0 if g % 3 == 0 else 1](dstt[:, g * 1024: (g + 1) * 1024], cp[:])

        def rope(n):
            """x = x*cos + partner(x)*sin on qc/kc (cols b,d,w); q on DVE,
            k on Pool."""
            dstt = qc if n == "q" else kc
            t1 = tmp.tile([128, QCOLS], BF16, tag="tmp")
            t2 = tmp.tile([128, QCOLS], BF16, tag="tmp")
            cos_in = _ap(costab, 0, [[32, D], [1, W], [0, BL]])
            x3 = _ap(dstt, 0, [[128, D], [4, W], [1, BL]])
            t1v = _ap(t1, 0, [[128, D], [4, W], [1, BL]])
            nc.vector.tensor_mul(t1v, x3, cos_in)
            part_in = _ap(dstt, 128, [[256, D // 2], [-128, 2], [4, W], [1, BL]])
            sin_in = _ap(sintab, 0, [[64, D // 2], [32, 2], [1, W], [0, BL]])
            t2v = _ap(t2, 0, [[256, D // 2], [128, 2], [4, W], [1, BL]])
            # the partner*sin product can run on Pool in parallel with DVE
            peng = nc.gpsimd if n == "k" else nc.vector
            peng.tensor_mul(t2v, part_in, sin_in)
            nc.vector.tensor_add(dstt[:], t1[:], t2[:])

        def transpose(n):
            """qc/kc [i, (d,w,b)] -> per-batch qt [64 d, (w,i)]."""
            srct = qc if n == "q" else kc
            for b in range(BL):
                dq = qt[(n, b)]
                for w4 in range(8):
                    tp = tps.tile([64, 512], BF16, tag="tp")
                    for wi in range(4):
                        w = w4 * 4 + wi
                        sv = _ap(srct, w * 4 + b, [[128, D]])
                        nc.tensor.transpose(
                            tp[:, wi * 128: (wi + 1) * 128], sv, ident[:])
                    drain_eng[w4 % 2](
                        dq[:, w4 * 512: (w4 + 1) * 512], tp[:])

        conv("q")
        rope("q")
        conv("k")
        rope("k")
        transpose("q")
        transpose("k")
        conv("v")
        if DEBUG:
            nc.sync.dma_start(out=dbg["qc"][:], in_=qc[:])
            nc.sync.dma_start(out=dbg["kc"][:], in_=kc[:])
            nc.sync.dma_start(out=dbg["vsb"][:], in_=vsb[:])
            nc.sync.dma_start(out=dbg["qt0"][:], in_=qt[("q", 0)][:])
            nc.sync.dma_start(out=dbg["kt0"][:], in_=qt[("k", 0)][:])

        tps.release()
        convps.release()
        tmp.release()
        tpool.release()
        xpool.release()

        # ---------------- attention phase
        simps = tc.alloc_tile_pool(name="simps", bufs=2, space="PSUM")
        avps = tc.alloc_tile_pool(name="avps", bufs=4, space="PSUM")
        ppool = tc.alloc_tile_pool(name="p", bufs=2)
        spool = tc.alloc_tile_pool(name="s", bufs=2)
        opool = tc.alloc_tile_pool(name="o", bufs=2)

        for b in range(BL):
            qtt = qt[("q", b)]
            ktt = qt[("k", b)]
            p = ppool.tile([128, PCOLS], BF16, tag="p")
            ot = opool.tile([128, OTC], BF16, tag="ot")
            # per 4-window group: sim -> exp -> mask -> AV -> drain, so the
            # engines pipeline across groups within a batch
            for t2 in range(8):
                sp = simps.tile([128, 1024], F32, tag="sp")
                ncols_t = 0
                for mi in range(4):
                    m = t2 * 4 + mi
                    ncols = 256 if m < W - 1 else 128
                    nc.tensor.matmul(
                        sp[:, mi * 256: mi * 256 + ncols],
                        ktt[:, m * 128: (m + 1) * 128],
                        qtt[:, m * 128: m * 128 + ncols],
                        start=True, stop=True,
                    )
                    ncols_t += ncols
                nc.scalar.activation(
                    p[:, t2 * 1024: t2 * 1024 + ncols_t],
                    sp[:, :ncols_t],
                    mybir.ActivationFunctionType.Exp,
                )
                if t2 == 0:
                    # key position 0 is pad-masked (torch quirk)
                    nc.vector.memset(p[0:1, 0:256], 0.0)
                # causal mask on own-window halves of this group
                meng = nc.gpsimd if t2 % 2 == 0 else nc.vector
                pview = _ap(p, t2 * 1024, [[256, 4], [1, 128]])
                tri_b = _ap(tri, 0, [[0, 4], [1, 128]])
                meng.tensor_mul(pview, pview, tri_b)
                if t2 == 0:
                    # window-0 query-0: uniform attention over own window
                    nc.vector.memset(p[:, 0:1], 1.0)

                av = avps.tile([128, 4 * 65], F32, tag="av",
                               padded_shape=[128, 512])
                for wi in range(4):
                    w = t2 * 4 + wi
                    slot = wi * 65
                    own = p[:, w * 256: w * 256 + 128]
                    vw = _ap(vsb, w * 4 + b, [[128, 65]])
                    if w == 0:
                        nc.tensor.matmul(av[:, slot: slot + 65], own, vw,
                                         start=True, stop=True)
                    else:
                        nc.tensor.matmul(av[:, slot: slot + 65], own, vw,
                                         start=True, stop=False)
                        prev = p[:, (w - 1) * 256 + 128: w * 256]
                        vprev = _ap(vsb, (w - 1) * 4 + b, [[128, 65]])
                        nc.tensor.matmul(av[:, slot: slot + 65], prev, vprev,
                                         start=False, stop=True)
                if t2 == 0:
                    # +128 phantom pad keys in the w0-q0 denominator
                    nc.vector.tensor_scalar_add(
                        av[0:1, 64:65], av[0:1, 64:65], 128.0)
                sr = spool.tile([128, 4], F32, tag="sr")
                nc.vector.reciprocal(sr[:], _ap(av, 64, [[65, 4]]))
                nc.vector.tensor_mul(
                    _ap(ot, t2 * 256, [[64, 4], [1, 64]]),
                    _ap(av, 0, [[65, 4], [1, 64]]),
                    _ap(sr, 0, [[1, 4], [0, 64]]),
                )
            if DEBUG and b == 0:
                nc.sync.dma_start(out=dbg["p0"][:], in_=p[:])
            nc.scalar.dma_start(
                out=bass.AP(tensor=out, offset=b * 128 * OTC,
                            ap=[[OTC, 128], [1, OTC]]),
                in_=ot[:])

        opool.release()
        spool.release()
        ppool.release()
        avps.release()
        simps.release()
        mid.release()
        const.release()

    _split_multiwaits(nc)
    return nc


_PROG = None


def _get_prog():
    global _PROG
    if _PROG is None:
        _PROG = _build_program()
    return _PROG


def _host_prep(q, k, v, wq, wk, wv):
    """Build per-core input maps (bf16 casts + layout/constant tables)."""
    jj = np.arange(WS)[:, None]   # jin
    ii = np.arange(WS)[None, :]   # jout
    hid = jj - ii + (WS - 1)      # hi: w[jin-jout+127] for jin <= jout
    lod = jj - ii - 1             # lo: w[jin-jout-1]   for jin >  jout
    him = (hid >= 0) & (hid < WS)
    lom = (lod >= 0) & (lod < WS)
    hidc = np.clip(hid, 0, WS - 1)
    lodc = np.clip(lod, 0, WS - 1)

    def toep(w, scale=1.0):
        wd = np.asarray(w, np.float32).reshape(D, WS) * scale
        # t[jin, d, {hi,lo}, jout]
        t = np.zeros((WS, D, 2, WS), np.float32)
        hi = np.where(him, wd[:, hidc], 0.0)          # [D, jin, jout]
        lo = np.where(lom, wd[:, lodc], 0.0)
        t[:, :, 0, :] = hi.transpose(1, 0, 2)
        t[:, :, 1, :] = lo.transpose(1, 0, 2)
        return np.ascontiguousarray(t.reshape(WS, TCOLS)).astype(NPBF)

    tq_np = toep(wq, SCALE)
    tk_np = toep(wk)
    tv_np = toep(wv)

    theta = 1.0 / ROPE_BASE ** (np.arange(0, D, 2, dtype=np.float32) / D)
    pm = np.arange(N, dtype=np.float32)[:, None] * theta[None, :]
    cos = np.repeat(np.cos(pm), 2, axis=-1)  # [n, d]
    sin = np.repeat(np.sin(pm), 2, axis=-1)
    sgn = np.where(np.arange(D) % 2 == 0, -1.0, 1.0).astype(np.float32)
    # [i, (d, w)] layout
    cosb_np = np.ascontiguousarray(
        cos.reshape(W, WS, D).transpose(1, 2, 0).reshape(WS, D * W)
    ).astype(NPBF)
    sinb_np = np.ascontiguousarray(
        (sin * sgn[None, :]).reshape(W, WS, D).transpose(1, 2, 0).reshape(WS, D * W)
    ).astype(NPBF)

    def xtile(x, sl):
        # [BL, N, D] f32 -> [128 i, (w'=33, b, d)] bf16 with w'=0 zeroed
        xb = np.asarray(x[sl], np.float32).astype(NPBF)
        buf = np.zeros((WS, NWP, BL, D), NPBF)
        buf[:, 1:] = xb.reshape(BL, W, WS, D).transpose(2, 1, 0, 3)
        return np.ascontiguousarray(buf.reshape(WS, XCOLS))

    in_maps = []
    for c in range(NCORES):
        sl = slice(c * BL, (c + 1) * BL)
        in_maps.append({
            "xq": xtile(q, sl), "xk": xtile(k, sl), "xv": xtile(v, sl),
            "tq": tq_np, "tk": tk_np, "tv": tv_np,
            "cosb": cosb_np, "sinb": sinb_np,
        })
    return in_maps


def _install_ntff_hook():
    """Provide antenv.axon_hooks with a ctypes NTFF profile hook (the slim
    container lacks it); enables trace=True under axon."""
    import sys as _sys
    import types
    import ctypes
    import contextlib

    try:
        from antenv.axon_hooks import get_axon_ntff_profile_hook  # noqa: F401
        return
    except ImportError:
        pass
    so_path = "/opt/axon/libaxon_pjrt.so"
    try:
        lib = ctypes.CDLL(so_path)
    except OSError:
        return
    if not hasattr(lib, "axon_start_nrt_profile"):
        return
    lib.axon_start_nrt_profile.argtypes = [
        ctypes.POINTER(ctypes.c_int64), ctypes.c_size_t]
    lib.axon_start_nrt_profile.restype = ctypes.c_int64
    lib.axon_stop_nrt_profile.argtypes = [ctypes.c_char_p]
    lib.axon_stop_nrt_profile.restype = ctypes.c_int64

    @contextlib.contextmanager
    def _hook(output_dir, device_ids):
        import jax
        jax.devices()
        if device_ids:
            ids = (ctypes.c_int64 * len(device_ids))(*device_ids)
            rc = lib.axon_start_nrt_profile(ids, len(device_ids))
        else:
            rc = lib.axon_start_nrt_profile(None, 0)
        if rc != 0:
            raise RuntimeError(f"axon_start_nrt_profile rc={rc}")
        try:
            yield
        finally:
            n = lib.axon_stop_nrt_profile(str(output_dir).encode())
            print(f"profile: {n} file(s) written to {output_dir}")

    import antenv

    mod = types.ModuleType("antenv.axon_hooks")
    _state = {"hook": _hook}
    mod.set_axon_ntff_profile_hook = lambda h: _state.__setitem__("hook", h)
    mod.get_axon_ntff_profile_hook = lambda: _state["hook"]
    _sys.modules["antenv.axon_hooks"] = mod
    antenv.axon_hooks = mod


def run(q, k, v, wq, wk, wv, trace=False):
    nc = _get_prog()
    in_maps = _host_prep(q, k, v, wq, wk, wv)
    if trace:
        _install_ntff_hook()
    res = run_bass_kernel_spmd(nc, in_maps, core_ids=list(range(NCORES)),
                               trace=trace)
    outs = []
    for c in range(NCORES):
        ob = np.asarray(res.results[c]["out"], np.float32)
        # [BL*128, (w, d)] -> [BL, N, D]
        outs.append(ob.reshape(BL, WS, W, D).transpose(0, 2, 1, 3)
                    .reshape(BL, N, D))
    outp = np.ascontiguousarray(np.concatenate(outs, axis=0))
    return outp, res


def kernel(q, k, v, wq, wk, wv):
    outp, _ = run(q, k, v, wq, wk, wv)
    return outp
